# revision 4
# baseline (speedup 1.0000x reference)
"""AttentionBlock (GroupNorm -> qkv 1x1 -> 4-head attention over 4096 tokens
-> proj 1x1 -> residual) distributed over 8 TRN2 NeuronCores.

Sharding: zero-communication query sharding. Core j handles batch b = j//2 and
query half qh = j%2 (2048 of the 4096 spatial positions). Each core loads the
full x[b] (256, 4096), computes GroupNorm + K/V over all keys, Q only for its
2048 queries, and writes its (256, 2048) output slice.

Structure (hybrid: pipelined prologue + oT-form attention):
  - scores are computed transposed ([keys, queries] via lhsT=k, rhs=q) so
    exp() output feeds the AV matmuls directly: oT accumulators keep queries
    on psum partitions, which makes the softmax normalization a per-partition
    reciprocal + scalar multiply (no cross-partition broadcast anywhere).
    The denominator rides as a ones column appended to V^T. Max-subtraction
    is skipped (logits are O(4) std-normal, far from fp32 exp overflow).
  - one exp() per [128 keys, 2 heads x 512 queries] psum tile; the Act
    engine is the bottleneck (only it can run Exp) and stays saturated
    through the four query chunks with no boundary stalls.
  - the prologue is fully pipelined: x arrives in chunks split across two
    DMA queues with bn_stats per chunk, the Exp table is preloaded, rsqrt
    uses the fp32 bit-trick + Newton steps (no Ln table load), junk matmuls
    hold the PE at boost clock through the stats tail, and K/V/Q production
    is interleaved into the first query chunk's key loop on a fixed slot
    schedule, using the transpose/projection psum banks (idle during the
    key loop) so the score-tile double buffer is never disturbed.
  - matmuls run in bf16 (full 128x128 PE, K=128 zero-padded per-head K
    tensors); GroupNorm stats, softmax accumulation/normalization and the
    residual stay fp32.
"""

import numpy as np

import concourse.bass as bass
import concourse.tile as tile
from concourse import bacc, mybir
from concourse.bass_utils import run_bass_kernel_spmd

C = 256
HW = 4096
NH = 4
D = 64  # head dim
G = 8  # groups
EPS = 1e-5
SCALE = D**-0.5
Q = HW // 2  # queries per core
NJT = HW // 128  # 32 key tiles
NKC = 8  # key chunks (512 keys each) for K/V production
NIC = Q // 512  # 4 query chunks of 512

F32 = mybir.dt.float32
BF16 = mybir.dt.bfloat16
I16 = mybir.dt.int16

# one-op exp on the DVE: exp(t) ~= bitcast_bf16(int16(A16*t + B16)) (Schraudolph
# in bf16 bit space; C=5.5 minimizes rms rel err ~1.8%, +0.5 compensates the
# truncating float->int convert). Softmax error is damped ~30x by the residual
# path, measured end-to-end rel err 6.9e-4 even with ALL tiles on this path.
A16 = 128.0 / float(np.log(2.0))
B16 = 127.0 * 128.0 - 5.5 + 0.5


def build(finalize=True):
    nc = bacc.Bacc("TRN2", target_bir_lowering=False, debug=False, num_devices=8)

    x = nc.declare_dram_parameter("x", [C, HW], BF16, isOutput=False)
    xq = nc.declare_dram_parameter("xq", [C, Q], F32, isOutput=False)
    wn2 = nc.declare_dram_parameter("wn2", [128, 2], F32, isOutput=False)
    bn2 = nc.declare_dram_parameter("bn2", [128, 2], F32, isOutput=False)
    wq = nc.declare_dram_parameter("wq", [128, 2, C], BF16, isOutput=False)
    bq2 = nc.declare_dram_parameter("bq2", [128, 2], F32, isOutput=False)
    wkz = nc.declare_dram_parameter("wkz", [128, 2, NH, 128], BF16, isOutput=False)
    bkz = nc.declare_dram_parameter("bkz", [128, NH], F32, isOutput=False)
    wv = nc.declare_dram_parameter("wv", [128, 2, NH * 65], BF16, isOutput=False)
    vb = nc.declare_dram_parameter("vb", [128, NH * 65], F32, isOutput=False)
    wproj = nc.declare_dram_parameter("wproj", [128, 2, C], BF16, isOutput=False)
    ident = nc.declare_dram_parameter("ident", [128, 128], BF16, isOutput=False)
    bproj2 = nc.declare_dram_parameter("bproj2", [128, 2], F32, isOutput=False)
    gmask = nc.declare_dram_parameter("gmask", [128, 2, 128], F32, isOutput=False)
    gmaskT = nc.declare_dram_parameter("gmaskT", [128, 2, 128], F32, isOutput=False)
    out = nc.declare_dram_parameter("out", [C, Q], F32, isOutput=True)

    Exp = mybir.ActivationFunctionType.Exp
    Ln = mybir.ActivationFunctionType.Ln
    Alu = mybir.AluOpType
    AVLAG = 6  # AV matmuls trail QK/exp by this many S tiles (2 per key tile)

    with tile.TileContext(nc) as tc:
        with (
            tc.tile_pool(name="keep", bufs=1) as keep,
            tc.tile_pool(name="consts", bufs=1) as consts,
            tc.tile_pool(name="small", bufs=4) as small,
            tc.tile_pool(name="s_ps", bufs=2, space="PSUM") as s_ps,
            tc.tile_pool(name="acc_ps", bufs=1, space="PSUM") as acc_ps,
            tc.tile_pool(name="exps", bufs=AVLAG + 2) as expp,
            tc.tile_pool(name="att", bufs=2) as att,
        ):
            # persistent attention operands
            KZ = [
                keep.tile([128, HW], BF16, tag=f"KZ{h}", name=f"KZ{h}")
                for h in range(NH)
            ]
            QT = [
                keep.tile([128, Q], BF16, tag=f"Q{t}", name=f"Q{t}")
                for t in range(2)
            ]
            # V^T with a leading ones column per head: [keys, (head, 1+d)]
            V = keep.tile([128, NJT, NH * 65], BF16)
            XQ = [
                keep.tile([128, Q], F32, tag=f"XQ{t}", name=f"XQ{t}")
                for t in range(2)
            ]

            with tc.tile_pool(name="xh", bufs=1) as xh:
                X = [
                    xh.tile([128, HW], BF16, tag=f"X{t}", name=f"X{t}")
                    for t in range(2)
                ]
                H = [
                    xh.tile([128, HW], BF16, tag=f"H{t}", name=f"H{t}")
                    for t in range(2)
                ]
                HQ = [
                    xh.tile([128, Q], BF16, tag=f"HQ{t}", name=f"HQ{t}")
                    for t in range(2)
                ]

                # preload the Exp activation table while DMAs run, and wake
                # the gpsimd firmware so its first real op pays no launch cost
                tldum = small.tile([1, 1], F32, tag="tld", name="tld", bufs=1)
                nc.vector.memset(tldum, 1.0)
                nc.scalar.activation(out=tldum, in_=tldum, func=Exp)
                gpdum = small.tile([1, 1], F32, tag="gpd", name="gpd", bufs=1)
                nc.gpsimd.memset(gpdum, 0.0)

                # ---- x DMA in chunks, bn_stats per chunk ----
                st = [
                    small.tile([128, 8, 6], F32, tag=f"bnst{t}", name=f"bnst{t}")
                    for t in range(2)
                ]
                JW = small.tile([128, 128], BF16, tag="junkw", name="junkw", bufs=1)
                JR = small.tile([128, 512], BF16, tag="junkr", name="junkr", bufs=1)
                nc.vector.memset(JW, 0.0)
                for ch in range(4):
                    for t in range(2):
                        eng = nc.sync if t == 0 else nc.scalar
                        eng.dma_start(
                            out=X[t][:, ch * 1024 : (ch + 1) * 1024],
                            in_=x[t * 128 : (t + 1) * 128, ch * 1024 : (ch + 1) * 1024],
                        )
                        xr = X[t].rearrange("p (n f) -> p n f", f=512)
                        for s in (2 * ch, 2 * ch + 1):
                            nc.vector.bn_stats(out=st[t][:, s], in_=xr[:, s])
                    if ch == 3:
                        # memset lands here in the DVE stream: the PE warmup
                        # below starts as stats wind down, so the array is
                        # still at full clock when the real matmuls arrive
                        nc.vector.memset(JR, 0.0)
                wps = s_ps.tile([128, 1024], F32, tag="sps", name="wps")
                for w in range(6):
                    nc.tensor.matmul(
                        out=wps[:, 0:512], lhsT=JW, rhs=JR, start=True, stop=True
                    )
                for ch in range(2):
                    for t in range(2):
                        eng = nc.sync if t == 0 else nc.scalar
                        eng.dma_start(
                            out=XQ[t][:, ch * 1024 : (ch + 1) * 1024],
                            in_=xq[
                                t * 128 : (t + 1) * 128,
                                ch * 1024 : (ch + 1) * 1024,
                            ],
                        )

                # ---- weights (after the x chunks in queue order) ----
                GM = consts.tile([128, 2, 128], F32)
                nc.sync.dma_start(out=GM, in_=gmask[:])
                GMT = consts.tile([128, 2, 128], F32)
                nc.scalar.dma_start(out=GMT, in_=gmaskT[:])
                WN = consts.tile([128, 2], F32)
                nc.sync.dma_start(out=WN, in_=wn2[:])
                BN = consts.tile([128, 2], F32)
                nc.scalar.dma_start(out=BN, in_=bn2[:])
                WQ = consts.tile([128, 2, C], BF16)
                nc.sync.dma_start(out=WQ, in_=wq[:])
                BQ = consts.tile([128, 2], F32)
                nc.scalar.dma_start(out=BQ, in_=bq2[:])
                WKZ = consts.tile([128, 2, NH, 128], BF16)
                nc.sync.dma_start(out=WKZ, in_=wkz[:])
                BKZ = consts.tile([128, NH], F32)
                nc.scalar.dma_start(out=BKZ, in_=bkz[:])
                WV = consts.tile([128, 2, NH * 65], BF16)
                nc.sync.dma_start(out=WV, in_=wv[:])
                VB = consts.tile([128, NH * 65], F32)
                nc.scalar.dma_start(out=VB, in_=vb[:])
                WP = consts.tile([128, 2, C], BF16)
                nc.sync.dma_start(out=WP, in_=wproj[:])
                IDENT = consts.tile([128, 128], BF16)
                nc.scalar.dma_start(out=IDENT, in_=ident[:])
                BP = consts.tile([128, 2], F32)
                nc.scalar.dma_start(out=BP, in_=bproj2[:])
                EPS8 = consts.tile([G, 1], F32)
                nc.vector.memset(EPS8, EPS)
                ONES65 = consts.tile([1, 65], F32)
                nc.vector.memset(ONES65, 1.0)

                # ---- GroupNorm statistics ----
                mv2 = small.tile([128, 2, 2], F32)  # [:, t, (mean, E[x^2])]
                for t in range(2):
                    mv = small.tile([128, 2], F32, tag="bnmv")
                    nc.vector.bn_aggr(out=mv, in_=st[t])
                    nc.vector.tensor_copy(out=mv2[:, t, 0:1], in_=mv[:, 0:1])
                    nc.vector.tensor_tensor(
                        out=mv2[:, t, 1:2], in0=mv[:, 0:1], in1=mv[:, 0:1],
                        op=Alu.mult,
                    )
                    nc.vector.tensor_tensor(
                        out=mv2[:, t, 1:2], in0=mv2[:, t, 1:2], in1=mv[:, 1:2],
                        op=Alu.add,
                    )

                gps = s_ps.tile([128, 1024], F32, tag="sps", name="gnps")
                for t in range(2):
                    nc.tensor.matmul(
                        out=gps[:, 0:2], lhsT=GM[:, t], rhs=mv2[:, t],
                        start=(t == 0), stop=(t == 1),
                    )
                gsb = small.tile([128, 2], F32)
                nc.vector.tensor_copy(out=gsb, in_=gps[:, 0:2])
                # gstat rows 0..8: col0 = group mean, col1 = rsqrt(var+eps);
                # rows 8..128 stay zero for the padded broadcast matmul.
                gstat = small.tile([128, 2], F32)
                nc.vector.memset(gstat, 0.0)
                nc.vector.tensor_copy(out=gstat[:G, 0:1], in_=gsb[:G, 0:1])
                gvar = small.tile([G, 1], F32)
                nc.vector.tensor_tensor(
                    out=gvar, in0=gsb[:G, 0:1], in1=gsb[:G, 0:1], op=Alu.mult
                )
                nc.vector.tensor_tensor(
                    out=gvar, in0=gsb[:G, 1:2], in1=gvar, op=Alu.subtract
                )
                # rsqrt via the fp32 bit-trick seed + 2 Newton steps (keeps
                # the Act engine free of Ln table loads)
                nc.vector.tensor_tensor(out=gvar, in0=gvar, in1=EPS8, op=Alu.add)
                gvi = gvar.bitcast(mybir.dt.int32)
                y0i = small.tile([G, 1], mybir.dt.int32, tag="y0i", name="y0i")
                nc.vector.tensor_scalar(
                    out=y0i, in0=gvi, scalar1=1, scalar2=0,
                    op0=Alu.logical_shift_right, op1=Alu.bitwise_or,
                )
                nc.vector.tensor_scalar(
                    out=y0i, in0=y0i, scalar1=-1, scalar2=0x5F3759DF,
                    op0=Alu.mult, op1=Alu.add,
                )
                y = y0i.bitcast(F32)
                yt = small.tile([G, 1], F32, tag="yt", name="yt")
                for _ in range(1):
                    nc.vector.tensor_tensor(out=yt, in0=y, in1=y, op=Alu.mult)
                    nc.vector.tensor_tensor(out=yt, in0=yt, in1=gvar, op=Alu.mult)
                    nc.vector.tensor_scalar(
                        out=yt, in0=yt, scalar1=-0.5, scalar2=1.5,
                        op0=Alu.mult, op1=Alu.add,
                    )
                    nc.vector.tensor_tensor(out=y, in0=y, in1=yt, op=Alu.mult)
                nc.vector.tensor_copy(out=gstat[:G, 1:2], in_=y)

                # broadcast group stats back to channels
                AB = []  # [t] -> [128, 2] (alpha, beta)
                for t in range(2):
                    bc = s_ps.tile([128, 1024], F32, tag="sps", name="bcst")
                    nc.tensor.matmul(out=bc[:, 0:2], lhsT=GMT[:, t], rhs=gstat)
                    bsb = small.tile([128, 2], F32, tag="bsb", name="bsb")
                    nc.vector.tensor_copy(out=bsb, in_=bc[:, 0:2])
                    ab = small.tile([128, 2], F32, tag=f"ab{t}", name=f"ab{t}")
                    # alpha = rstd * w
                    nc.vector.tensor_tensor(
                        out=ab[:, 0:1], in0=bsb[:, 1:2], in1=WN[:, t : t + 1],
                        op=Alu.mult,
                    )
                    # beta = b - mean * alpha
                    nc.vector.tensor_tensor(
                        out=ab[:, 1:2], in0=bsb[:, 0:1], in1=ab[:, 0:1],
                        op=Alu.mult,
                    )
                    nc.vector.tensor_tensor(
                        out=ab[:, 1:2], in0=BN[:, t : t + 1], in1=ab[:, 1:2],
                        op=Alu.subtract,
                    )
                    AB.append(ab)

                # ---- chunked production helpers ----
                def hq_chunk(c):  # normalized queries, 512 cols (DVE)
                    for t in range(2):
                        nc.vector.tensor_scalar(
                            out=HQ[t][:, c * 512 : (c + 1) * 512],
                            in0=XQ[t][:, c * 512 : (c + 1) * 512],
                            scalar1=AB[t][:, 0:1], scalar2=AB[t][:, 1:2],
                            op0=Alu.mult, op1=Alu.add,
                        )

                def h_chunk(c):  # normalized keys, 512 cols (gpsimd: pure
                    # SBUF->SBUF, keeps the DVE free for psum drains)
                    for t in range(2):
                        nc.gpsimd.tensor_scalar(
                            out=H[t][:, c * 512 : (c + 1) * 512],
                            in0=X[t][:, c * 512 : (c + 1) * 512],
                            scalar1=AB[t][:, 0:1], scalar2=AB[t][:, 1:2],
                            op0=Alu.mult, op1=Alu.add,
                        )

                def q_chunk(c):  # q projection for queries 512c.. (both t)
                    for t in range(2):
                        ps = acc_ps.tile(
                            [128, 512], F32,
                            tag="trps" if t == 0 else "pjps", name="qps",
                        )
                        for ct in range(2):
                            nc.tensor.matmul(
                                out=ps,
                                lhsT=WQ[:, ct, t * 128 : (t + 1) * 128],
                                rhs=HQ[ct][:, c * 512 : (c + 1) * 512],
                                start=(ct == 0), stop=(ct == 1),
                            )
                        nc.vector.tensor_scalar_add(
                            out=QT[t][:, c * 512 : (c + 1) * 512],
                            in0=ps, scalar1=BQ[:, t : t + 1],
                        )

                def k_piece(n, hp):  # K head-pair hp for keys 512n..
                    for h2 in range(2):
                        h = 2 * hp + h2
                        ps = acc_ps.tile(
                            [128, 512], F32,
                            tag="trps" if h2 == 0 else "pjps", name="kps",
                        )
                        for ct in range(2):
                            nc.tensor.matmul(
                                out=ps,
                                lhsT=WKZ[:, ct, h],
                                rhs=H[ct][:, n * 512 : (n + 1) * 512],
                                start=(ct == 0), stop=(ct == 1),
                            )
                        nc.vector.tensor_scalar_add(
                            out=KZ[h][:, n * 512 : (n + 1) * 512],
                            in0=ps, scalar1=BKZ[:, h : h + 1],
                        )

                def v_piece(n, jp):  # V^T for key tiles 4n+2jp, 4n+2jp+1
                    for jo in range(2):
                        j = 4 * n + 2 * jp + jo
                        ps = acc_ps.tile(
                            [128, 512], F32,
                            tag="trps" if jo == 0 else "pjps", name="vps",
                        )
                        for ct in range(2):
                            nc.tensor.matmul(
                                out=ps[:, 0 : NH * 65],
                                lhsT=H[ct][:, j * 128 : (j + 1) * 128],
                                rhs=WV[:, ct],
                                start=(ct == 0), stop=(ct == 1),
                            )
                        nc.vector.tensor_tensor(
                            out=V[:, j],
                            in0=ps[:, 0 : NH * 65],
                            in1=VB, op=Alu.add,
                        )

                # minimal chain to the first QK: chunk 0 of HQ/H/Q/K
                # (the hp=1 head-pair K arrives via the hp0 slot schedule)
                hq_chunk(0)
                h_chunk(0)
                h_chunk(1)
                q_chunk(0)
                k_piece(0, 0)

                # ic0 production schedule: jp slot -> tasks, per hp.
                # V chunk m lands at slot 2m (just in time for its own AVs),
                # K chunk m+1 and H chunk m+2 at slot 2m+1; hp1 only needs
                # its own K head-pair. q/hq chunks ride along for later ics.
                prod0, prod1 = {}, {}
                prod0[0] = [lambda: v_piece(0, 0), lambda: v_piece(0, 1)]
                for m in range(1, NKC):
                    tasks = []
                    if m + 1 < NKC:
                        tasks.append(lambda c=m + 1: h_chunk(c))
                    tasks.append(lambda c=m: k_piece(c, 0))
                    prod0[2 * m - 1] = tasks
                    prod0[2 * m] = [
                        lambda c=m: v_piece(c, 0), lambda c=m: v_piece(c, 1)
                    ]
                for qi, p in ((1, 4), (2, 8), (3, 12)):
                    prod0[p] = prod0.get(p, []) + [
                        lambda c=qi: hq_chunk(c), lambda c=qi: q_chunk(c)
                    ]
                prod0[14] = prod0.get(14, []) + [lambda: k_piece(0, 1)]
                for m in range(1, NKC):
                    prod1[2 * m - 1] = [lambda c=m: k_piece(c, 1)]

                # ---- attention + projection (oT form: queries on psum
                # partitions, exp(scores) streamed as the stationary operand,
                # per-partition softmax normalization) ----
                # exp tiles are split between the Act engine (native Exp) and
                # the DVE (int16 Schraudolph, one tensor_scalar): Act alone is
                # the bottleneck at ~285us; the DVE has slack. ic0 routes
                # fewer tiles to the DVE (it is busy with K/V/Q drains there).
                exp_cnt = [0]

                def do_exp(S, name):
                    E = expp.tile([128, 1024], I16, tag="exps", name=name)
                    i = exp_cnt[0]
                    exp_cnt[0] += 1
                    num = 16 if i < 64 else 26  # DVE tiles per 64
                    if (i % 64) * num % 64 < num:
                        nc.vector.tensor_scalar(
                            out=E, in0=S, scalar1=A16 * SCALE, scalar2=B16,
                            op0=Alu.mult, op1=Alu.add,
                        )
                    else:
                        nc.scalar.activation(
                            out=E.bitcast(BF16), in_=S, func=Exp, scale=SCALE
                        )
                    return E.bitcast(BF16)

                for ic in range(NIC):
                    oTn = [
                        att.tile([128, C], BF16, tag=f"oTn{isub}", name=f"oTn{isub}")
                        for isub in range(4)
                    ]
                    OSB = [
                        att.tile([128, 512], BF16, tag=f"osb{ct}", name=f"osb{ct}")
                        for ct in range(2)
                    ]
                    for hp in range(2):
                        oT = [
                            acc_ps.tile(
                                [128, 4, 68], F32, tag=f"ot{h2}", name=f"ot{h2}"
                            )
                            for h2 in range(2)
                        ]

                        def qk_into(S, j):
                            for h2 in range(2):
                                nc.tensor.matmul(
                                    out=S[:, h2 * 512 : (h2 + 1) * 512],
                                    lhsT=KZ[2 * hp + h2][
                                        :, j * 128 : (j + 1) * 128
                                    ],
                                    rhs=QT[hp][:, ic * 512 : (ic + 1) * 512],
                                    start=True, stop=True,
                                )

                        def av_from(E, j):
                            for h2 in range(2):
                                head = 2 * hp + h2
                                for isub in range(4):
                                    nc.tensor.matmul(
                                        out=oT[h2][:, isub, 0:65],
                                        lhsT=E[
                                            :,
                                            h2 * 512 + isub * 128 : h2 * 512
                                            + (isub + 1) * 128,
                                        ],
                                        rhs=V[:, j, head * 65 : (head + 1) * 65],
                                        start=(j == 0 and isub == 0),
                                        stop=(j == NJT - 1 and isub == 3),
                                    )

                        for jp in range(NJT // 2):
                            if ic == 0:
                                sched = prod0 if hp == 0 else prod1
                                for task in sched.get(jp, ()):
                                    task()
                            j0, j1 = 2 * jp, 2 * jp + 1
                            S0 = s_ps.tile([128, 1024], F32, tag="sps", name="s0")
                            qk_into(S0, j0)
                            S1 = s_ps.tile([128, 1024], F32, tag="sps", name="s1")
                            qk_into(S1, j1)
                            E0 = do_exp(S0, "e0")
                            E1 = do_exp(S1, "e1")
                            av_from(E0, j0)
                            av_from(E1, j1)
                        # normalize by the ones-column sums (per-partition;
                        # one strided reciprocal covers all 4 sub-tiles)
                        for h2 in range(2):
                            head = 2 * hp + h2
                            r4 = small.tile([128, 4], F32, tag="recip", name="recip")
                            nc.vector.reciprocal(
                                out=r4,
                                in_=oT[h2][:, :, 64:65].rearrange(
                                    "p a b -> p (a b)"
                                ),
                            )
                            for isub in range(4):
                                nc.vector.tensor_scalar_mul(
                                    out=oTn[isub][:, head * 64 : (head + 1) * 64],
                                    in0=oT[h2][:, isub, 0:64],
                                    scalar1=r4[:, isub : isub + 1],
                                )
                    # transpose oTn -> [channels, 512 queries]
                    for ct in range(2):
                        for isub in range(4):
                            tp = acc_ps.tile(
                                [128, 128], BF16, tag="trps", name="trps"
                            )
                            nc.tensor.transpose(
                                tp, oTn[isub][:, ct * 128 : (ct + 1) * 128], IDENT
                            )
                            nc.vector.tensor_copy(
                                out=OSB[ct][:, isub * 128 : (isub + 1) * 128],
                                in_=tp,
                            )
                    # proj + bias + residual
                    for mt in range(2):
                        ps = acc_ps.tile([128, 512], F32, tag="pjps", name="pjps")
                        for ct in range(2):
                            nc.tensor.matmul(
                                out=ps,
                                lhsT=WP[:, ct, mt * 128 : (mt + 1) * 128],
                                rhs=OSB[ct],
                                start=(ct == 0), stop=(ct == 1),
                            )
                        ob = att.tile([128, 512], F32, tag="outsb", name="outsb")
                        nc.vector.scalar_tensor_tensor(
                            out=ob, in0=ps, scalar=BP[:, mt : mt + 1],
                            in1=XQ[mt][:, ic * 512 : (ic + 1) * 512],
                            op0=Alu.add, op1=Alu.add,
                        )
                        nc.sync.dma_start(
                            out=out[
                                mt * 128 : (mt + 1) * 128,
                                ic * 512 : (ic + 1) * 512,
                            ],
                            in_=ob,
                        )
    if finalize:
        nc.finalize()
    return nc


def _prep_weights(norm_w, norm_b, qkv_w, qkv_b, proj_w, proj_b):
    """Host-side layout (pure reshapes/transposes + dtype casts of weights)."""
    import ml_dtypes

    f = np.float32
    cdt = ml_dtypes.bfloat16

    def ctile(v):  # (256,) -> (128, 2) per channel-tile columns
        return np.ascontiguousarray(np.asarray(v).reshape(2, 128).T, dtype=f)

    def ptile(m):  # (256, N) -> (128, 2, N)
        return np.ascontiguousarray(
            np.asarray(m).reshape(2, 128, -1).transpose(1, 0, 2), dtype=f
        )

    qkv_w = np.asarray(qkv_w)
    qkv_b = np.asarray(qkv_b)
    wqT = qkv_w[:C].T  # (256, 256)
    wkT = qkv_w[C : 2 * C].T  # (256, 256) key rows
    # per-head K weights, zero-padded so each head's output occupies the same
    # 64 partition rows as its q in the packed 2-head Q tile
    wkzT = np.zeros((C, NH, 128), dtype=f)
    bkz = np.zeros((128, NH), dtype=f)
    for h in range(NH):
        off = 64 * (h % 2)
        wkzT[:, h, off : off + 64] = wkT[:, h * 64 : (h + 1) * 64]
        bkz[off : off + 64, h] = qkv_b[C + h * 64 : C + (h + 1) * 64]
    wvm = qkv_w[2 * C :]  # (256, 256)
    wvT = np.zeros((C, NH * 65), dtype=f)
    vb = np.zeros((128, NH * 65), dtype=f)
    for h in range(NH):
        wvT[:, h * 65 : h * 65 + 64] = wvm[h * 64 : (h + 1) * 64].T
        vb[:, h * 65 : h * 65 + 64] = qkv_b[
            2 * C + h * 64 : 2 * C + (h + 1) * 64
        ][None, :]
        vb[:, h * 65 + 64] = 1.0  # ones column -> denominator at oT column 64
    # zero-padded group masks (value 1/32 for group-mean aggregation; one-hot
    # transpose for the broadcast back to channels)
    gm = np.zeros((C, 128), dtype=f)
    for c in range(C):
        gm[c, c // 32] = 1.0 / 32.0
    # gmaskT param layout [p, t, 128]: partition p = group index (only 0..8
    # nonzero), free = channel within tile t
    gmaskT = np.zeros((128, 2, 128), dtype=f)
    for c in range(C):
        gmaskT[c // 32, c // 128, c % 128] = 1.0

    return dict(
        wn2=ctile(norm_w),
        bn2=ctile(norm_b),
        wq=ptile(wqT).astype(cdt),
        bq2=np.ascontiguousarray(qkv_b[:C].reshape(2, 128).T, dtype=f),
        wkz=ptile(wkzT.reshape(C, NH * 128))
        .reshape(128, 2, NH, 128)
        .astype(cdt),
        bkz=bkz,
        wv=ptile(wvT).astype(cdt),
        vb=vb,
        wproj=ptile(np.asarray(proj_w).T).astype(cdt),
        ident=np.eye(128, dtype=cdt),
        bproj2=ctile(proj_b),
        gmask=ptile(gm),
        gmaskT=gmaskT,
    )


_NC_CACHE = {}
_RUN_OPTS = {}  # extra kwargs for run_bass_kernel_spmd (test harness sets trace)
LAST_RESULT = None


def _get_nc():
    if "nc" not in _NC_CACHE:
        _NC_CACHE["nc"] = build()
    return _NC_CACHE["nc"]


def kernel(x, norm_w, norm_b, qkv_w, qkv_b, proj_w, proj_b, **_):
    import ml_dtypes

    nc = _get_nc()
    w = _prep_weights(norm_w, norm_b, qkv_w, qkv_b, proj_w, proj_b)
    x = np.asarray(x, dtype=np.float32)
    Bv, Cv, Hv, Wv = x.shape
    xf = x.reshape(Bv, Cv, Hv * Wv)
    # x feeds GroupNorm stats + K/V (bf16 math downstream anyway): ship it
    # bf16 to halve the stats-gating DMA; the residual path keeps xq in f32
    xb = xf.astype(ml_dtypes.bfloat16)
    in_maps = []
    for j in range(8):
        b, qh = j // 2, j % 2
        m = dict(w)
        m["x"] = np.ascontiguousarray(xb[b])
        m["xq"] = np.ascontiguousarray(xf[b][:, qh * Q : (qh + 1) * Q])
        in_maps.append(m)
    res = run_bass_kernel_spmd(nc, in_maps, core_ids=list(range(8)), **_RUN_OPTS)
    global LAST_RESULT
    LAST_RESULT = res
    outf = np.empty((Bv, Cv, Hv * Wv), dtype=np.float32)
    for j in range(8):
        b, qh = j // 2, j % 2
        outf[b][:, qh * Q : (qh + 1) * Q] = res.results[j]["out"]
    return outf.reshape(Bv, Cv, Hv, Wv)

